# revision 5
# baseline (speedup 1.0000x reference)
"""Trainium2 Bass kernel for the variants-attention module.

Model (reference):
    q = (x @ Wq)                          [B,N,H,D]
    kv = variants @ Wkv -> k,v            [V,B,N,H,D] each
    attn = softmax(q.k / sqrt(D)) over V  (per-token attention over variants)
    out = (attn.v) @ Wp + bp              [B,N,C]

Strategy: data-parallel over the B*N = 16384 tokens across 8 NeuronCores
(2048 tokens/core), weights replicated.  Host pre-casts inputs to bf16 and
pre-transposes activations to feature-major so the kernel streams them into
the PE array without on-chip transposes.  Projections run on the tensor
engine in bf16 (fp32 PSUM accumulate); the tiny per-token attention over
V=4 variants runs on the vector engine in bf16; softmax exp on the scalar
engine; the attended output is transposed back with PE-transpose and
projected through Wp (+bp folded in as a K=1 matmul row).
"""

import numpy as np
import ml_dtypes

import concourse.bass as bass
import concourse.bacc as bacc
import concourse.tile as tile
from concourse import mybir
from concourse.bass_utils import run_bass_kernel_spmd

# ---------------------------------------------------------------------------

V, B, N, C, H = 4, 4, 4096, 768, 12
D = C // H
SCALE = D**-0.5
TOK = B * N
N_CORES = 8
TPC = TOK // N_CORES  # tokens per core

BF16 = mybir.dt.bfloat16
F32 = mybir.dt.float32
CK = C // 128  # 6 feature chunks

nbf16 = ml_dtypes.bfloat16


def build_nc(tpc=TPC, tile_tok=512, repeat=1):
    """Build the per-core Bass program for `tpc` tokens.

    repeat>1 re-runs the whole computation that many times (idempotent
    output writes) — used only for timing, so the ~100ms axon dispatch
    floor can be differenced away: exec = (T(R) - T(1)) / (R - 1).
    """
    assert tpc % tile_tok == 0 and tile_tok % 128 == 0
    n_tiles = tpc // tile_tok
    n_ch = tile_tok // 128  # 128-token chunks per tile

    nc = bacc.Bacc("TRN2", target_bir_lowering=False, debug=False, num_devices=N_CORES)

    xT = nc.dram_tensor("xT", [C, tpc], BF16, kind="ExternalInput").ap()
    pT = nc.dram_tensor("pT", [V, C, tpc], BF16, kind="ExternalInput").ap()
    wq = nc.dram_tensor("wq", [C, C], BF16, kind="ExternalInput").ap()
    wkv = nc.dram_tensor("wkv", [C, 2 * C], BF16, kind="ExternalInput").ap()
    wp = nc.dram_tensor("wp", [C, C], BF16, kind="ExternalInput").ap()
    bp = nc.dram_tensor("bp", [1, C], BF16, kind="ExternalInput").ap()
    ident = nc.dram_tensor("ident", [128, 128], BF16, kind="ExternalInput").ap()
    out = nc.dram_tensor("out", [tpc, C], F32, kind="ExternalOutput").ap()

    xT_v = xT.rearrange("(ck p) t -> p ck t", p=128)
    pT_v = pT.rearrange("v (ck p) t -> p v ck t", p=128)

    with tile.TileContext(nc) as tc:
        with (
            tc.tile_pool(name="const", bufs=1) as constp,
            tc.tile_pool(name="xin", bufs=2) as xin,
            tc.tile_pool(name="pin", bufs=2) as pin,
            tc.tile_pool(name="qkv", bufs=2) as qkvp,
            tc.tile_pool(name="attn", bufs=2) as attp,
            tc.tile_pool(name="outs", bufs=2) as outp,
            tc.tile_pool(name="proj", bufs=2, space="PSUM") as projp,
            tc.tile_pool(name="tps", bufs=2, space="PSUM") as tpsp,
        ):
            # --- persistent constants ---
            wq_sb = constp.tile([128, CK, C], BF16, tag="wq")
            nc.sync.dma_start(wq_sb[:], wq.rearrange("(ck p) o -> p ck o", p=128))
            wkv_sb = constp.tile([128, CK, 2 * C], BF16, tag="wkv")
            nc.sync.dma_start(wkv_sb[:], wkv.rearrange("(ck p) o -> p ck o", p=128))
            wp_sb = constp.tile([128, CK, C], BF16, tag="wp")
            nc.sync.dma_start(wp_sb[:], wp.rearrange("(ck p) o -> p ck o", p=128))
            bp_sb = constp.tile([1, C], BF16, tag="bp")
            nc.sync.dma_start(bp_sb[:], bp[:])
            id_sb = constp.tile([128, 128], BF16, tag="ident")
            nc.sync.dma_start(id_sb[:], ident[:])
            ones_sb = constp.tile([1, 128], BF16, tag="ones")
            nc.gpsimd.memset(ones_sb[:], 1.0)

            for rep in range(repeat):
              for it in range(n_tiles):
                t0 = it * tile_tok
                xt = xin.tile([128, CK, tile_tok], BF16, tag="xt")
                nc.sync.dma_start(xt[:], xT_v[:, :, t0 : t0 + tile_tok])
                pt = pin.tile([128, V, CK, tile_tok], BF16, tag="pt")
                nc.sync.dma_start(pt[:], pT_v[:, :, :, t0 : t0 + tile_tok])

                for tc_i in range(n_ch):
                    ts = slice(tc_i * 128, (tc_i + 1) * 128)
                    row0 = t0 + tc_i * 128

                    # ---- q projection: q[t, C] (token-partition) ----
                    q_ps = projp.tile([128, C], F32, tag="proj")
                    for ck in range(CK):
                        lhsT = xt[:, ck, ts]
                        nc.tensor.matmul(
                            q_ps[:, 0:512], lhsT, wq_sb[:, ck, 0:512],
                            start=(ck == 0), stop=(ck == CK - 1),
                        )
                        nc.tensor.matmul(
                            q_ps[:, 512:768], lhsT, wq_sb[:, ck, 512:768],
                            start=(ck == 0), stop=(ck == CK - 1),
                        )
                    q_sb = qkvp.tile([128, C], BF16, tag="q")
                    nc.scalar.copy(q_sb[:], q_ps[:])

                    # ---- kv projections per variant ----
                    k_sbs, v_sbs = [], []
                    for v in range(V):
                        kv_ps = projp.tile([128, 2 * C], F32, tag="proj")
                        for ck in range(CK):
                            lhsT = pt[:, v, ck, ts]
                            for co in range(3):
                                nc.tensor.matmul(
                                    kv_ps[:, co * 512 : (co + 1) * 512],
                                    lhsT,
                                    wkv_sb[:, ck, co * 512 : (co + 1) * 512],
                                    start=(ck == 0), stop=(ck == CK - 1),
                                )
                        k_sb = qkvp.tile([128, C], BF16, tag=f"k{v}")
                        v_sb = qkvp.tile([128, C], BF16, tag=f"v{v}")
                        if v % 2 == 0:
                            nc.scalar.copy(k_sb[:], kv_ps[:, 0:C])
                            nc.vector.tensor_copy(v_sb[:], kv_ps[:, C : 2 * C])
                        else:
                            nc.vector.tensor_copy(k_sb[:], kv_ps[:, 0:C])
                            nc.scalar.copy(v_sb[:], kv_ps[:, C : 2 * C])
                        k_sbs.append(k_sb)
                        v_sbs.append(v_sb)

                    # ---- logits: L[t, v, h] = SCALE * sum_d q*k ----
                    L = attp.tile([128, V, H], F32, tag="logits")
                    for v in range(V):
                        prod = attp.tile([128, C], BF16, tag="prod")
                        nc.vector.scalar_tensor_tensor(
                            prod[:], q_sb[:], SCALE, k_sbs[v][:],
                            op0=mybir.AluOpType.mult, op1=mybir.AluOpType.mult,
                        )
                        nc.vector.tensor_reduce(
                            L[:, v, :],
                            prod[:].rearrange("p (h d) -> p h d", d=D),
                            axis=mybir.AxisListType.X,
                            op=mybir.AluOpType.add,
                        )

                    # ---- softmax over v ----
                    E = attp.tile([128, V, H], F32, tag="exps")
                    nc.scalar.activation(E[:], L[:], mybir.ActivationFunctionType.Exp)
                    s01 = attp.tile([128, H], F32, tag="s01")
                    s23 = attp.tile([128, H], F32, tag="s23")
                    ssum = attp.tile([128, H], F32, tag="ssum")
                    nc.vector.tensor_add(s01[:], E[:, 0, :], E[:, 1, :])
                    nc.vector.tensor_add(s23[:], E[:, 2, :], E[:, 3, :])
                    nc.vector.tensor_add(ssum[:], s01[:], s23[:])
                    rcp = attp.tile([128, H], F32, tag="rcp")
                    nc.vector.reciprocal(rcp[:], ssum[:])
                    W = attp.tile([128, V, H], BF16, tag="wgt")
                    for v in range(V):
                        nc.vector.tensor_mul(W[:, v, :], E[:, v, :], rcp[:])

                    # ---- attended = sum_v w_v * v_v  (w broadcast over d) ----
                    tmp = []
                    for v in range(V):
                        tv = attp.tile([128, C], BF16, tag=f"tv{v}")
                        wb = W[:, v, :].unsqueeze(-1).broadcast_to([128, H, D])
                        nc.vector.tensor_mul(
                            tv[:].rearrange("p (h d) -> p h d", d=D),
                            v_sbs[v][:].rearrange("p (h d) -> p h d", d=D),
                            wb,
                        )
                        tmp.append(tv)
                    a01 = attp.tile([128, C], BF16, tag="a01")
                    a23 = attp.tile([128, C], BF16, tag="a23")
                    att = attp.tile([128, C], BF16, tag="att")
                    nc.vector.tensor_add(a01[:], tmp[0][:], tmp[1][:])
                    nc.vector.tensor_add(a23[:], tmp[2][:], tmp[3][:])
                    nc.vector.tensor_add(att[:], a01[:], a23[:])

                    # ---- transpose attended to feature-partition ----
                    attT = outp.tile([128, CK, 128], BF16, tag="attT")
                    for ck in range(CK):
                        tp = tpsp.tile([128, 128], BF16, tag="tps")
                        nc.tensor.transpose(tp[:], att[:, ck * 128 : (ck + 1) * 128], id_sb[:])
                        nc.vector.tensor_copy(attT[:, ck, :], tp[:])

                    # ---- output projection + bias ----
                    o_ps = projp.tile([128, C], F32, tag="proj")
                    for ck in range(CK):
                        lhsT = attT[:, ck, :]
                        nc.tensor.matmul(
                            o_ps[:, 0:512], lhsT, wp_sb[:, ck, 0:512],
                            start=(ck == 0), stop=False,
                        )
                        nc.tensor.matmul(
                            o_ps[:, 512:768], lhsT, wp_sb[:, ck, 512:768],
                            start=(ck == 0), stop=False,
                        )
                    nc.tensor.matmul(
                        o_ps[:, 0:512], ones_sb[:], bp_sb[:, 0:512],
                        start=False, stop=True,
                    )
                    nc.tensor.matmul(
                        o_ps[:, 512:768], ones_sb[:], bp_sb[:, 512:768],
                        start=False, stop=True,
                    )
                    o_sb = outp.tile([128, C], F32, tag="osb")
                    nc.scalar.copy(o_sb[:], o_ps[:])
                    nc.sync.dma_start(out[row0 : row0 + 128, :], o_sb[:])

    nc.compile()
    return nc


def _prep_inputs(x, variants_patches, Wq, Wkv, Wp, bp):
    """Host-side: cast to bf16, transpose activations feature-major, shard."""
    xs = np.ascontiguousarray(x.reshape(TOK, C).T.astype(nbf16))  # [C, TOK]
    ps = np.ascontiguousarray(
        variants_patches.reshape(V, TOK, C).transpose(0, 2, 1).astype(nbf16)
    )  # [V, C, TOK]
    wq = np.ascontiguousarray(Wq.astype(nbf16))
    wkv = np.ascontiguousarray(Wkv.astype(nbf16))
    wp = np.ascontiguousarray(Wp.astype(nbf16))
    bpb = np.ascontiguousarray(bp.reshape(1, C).astype(nbf16))
    ident = np.eye(128, dtype=nbf16)

    in_maps = []
    for c in range(N_CORES):
        sl = slice(c * TPC, (c + 1) * TPC)
        in_maps.append(
            {
                "xT": np.ascontiguousarray(xs[:, sl]),
                "pT": np.ascontiguousarray(ps[:, :, sl]),
                "wq": wq,
                "wkv": wkv,
                "wp": wp,
                "bp": bpb,
                "ident": ident,
            }
        )
    return in_maps


_NC_CACHE = {}


def run(x, variants_patches, Wq, Wkv, Wp, bp, **spmd_kwargs):
    if "nc" not in _NC_CACHE:
        _NC_CACHE["nc"] = build_nc()
    nc = _NC_CACHE["nc"]
    in_maps = _prep_inputs(x, variants_patches, Wq, Wkv, Wp, bp)
    res = run_bass_kernel_spmd(nc, in_maps, core_ids=list(range(N_CORES)), **spmd_kwargs)
    full = np.concatenate([res.results[c]["out"] for c in range(N_CORES)], axis=0)
    return full.reshape(B, N, C), res


def bench(nc, in_maps, iters=20):
    """Repeated-execution timing of the SPMD NEFF via the PJRT path.

    Returns (per_iter_seconds_list, results_of_last_iter). Inputs live on
    device across iterations; each iteration re-donates freshly-uploaded
    zero output buffers.
    """
    import jax
    import time
    from jax.sharding import Mesh, PartitionSpec
    from jax.experimental.shard_map import shard_map
    from concourse import bass2jax, mybir as _mybir
    from concourse.bass2jax import _bass_exec_p, install_neuronx_cc_hook

    install_neuronx_cc_hook()
    n_cores = len(in_maps)
    partition_name = nc.partition_id_tensor.name if nc.partition_id_tensor else None

    in_names, out_names, out_avals, zero_outs = [], [], [], []
    for alloc in nc.m.functions[0].allocations:
        if not isinstance(alloc, _mybir.MemoryLocationSet):
            continue
        name = alloc.memorylocations[0].name
        if alloc.kind == "ExternalInput":
            if name != partition_name:
                in_names.append(name)
        elif alloc.kind == "ExternalOutput":
            shape = tuple(alloc.tensor_shape)
            dtype = _mybir.dt.np(alloc.dtype)
            out_names.append(name)
            out_avals.append(jax.core.ShapedArray(shape, dtype))
            zero_outs.append(np.zeros(shape, dtype))
    n_params = len(in_names)
    n_outs = len(out_avals)
    in_names_all = in_names + out_names
    if partition_name is not None:
        in_names_all.append(partition_name)

    def _body(*args):
        operands = list(args)
        if partition_name is not None:
            operands.append(bass2jax.partition_id_tensor())
        outs = _bass_exec_p.bind(
            *operands,
            out_avals=tuple(out_avals),
            in_names=tuple(in_names_all),
            out_names=tuple(out_names),
            lowering_input_output_aliases=(),
            sim_require_finite=True,
            sim_require_nnan=True,
            nc=nc,
        )
        return tuple(outs)

    devices = jax.devices()[:n_cores]
    mesh = Mesh(np.asarray(devices), ("core",))
    donate = tuple(range(n_params, n_params + n_outs))
    sharded = jax.jit(
        shard_map(
            _body, mesh=mesh,
            in_specs=(PartitionSpec("core"),) * (n_params + n_outs),
            out_specs=(PartitionSpec("core"),) * n_outs,
            check_rep=False,
        ),
        donate_argnums=donate, keep_unused=True,
    )
    sh = jax.sharding.NamedSharding(mesh, PartitionSpec("core"))
    concat_in = [
        jax.device_put(
            np.concatenate([np.asarray(in_maps[c][nm]) for c in range(n_cores)], axis=0),
            sh,
        )
        for nm in in_names
    ]
    def fresh_zeros():
        return [
            jax.device_put(np.zeros((n_cores * z.shape[0], *z.shape[1:]), z.dtype), sh)
            for z in zero_outs
        ]

    # warmup / compile
    out = sharded(*concat_in, *fresh_zeros())
    jax.block_until_ready(out)

    times = []
    for _ in range(iters):
        zs = fresh_zeros()
        jax.block_until_ready(zs)
        t0 = time.perf_counter()
        out = sharded(*concat_in, *zs)
        jax.block_until_ready(out)
        times.append(time.perf_counter() - t0)

    results = [
        {nm: np.asarray(out[i]).reshape(n_cores, *out_avals[i].shape)[c]
         for i, nm in enumerate(out_names)}
        for c in range(n_cores)
    ]
    return times, results


def kernel(x, variants_patches, num_layer=None, Wq=None, Wkv=None, Wp=None, bp=None):
    x = np.asarray(x, dtype=np.float32)
    variants_patches = np.asarray(variants_patches, dtype=np.float32)
    Wq = np.asarray(Wq, dtype=np.float32)
    Wkv = np.asarray(Wkv, dtype=np.float32)
    Wp = np.asarray(Wp, dtype=np.float32)
    bp = np.asarray(bp, dtype=np.float32)
    out, _ = run(x, variants_patches, Wq, Wkv, Wp, bp)
    return out


# revision 13
# speedup vs baseline: 1.5467x; 1.5467x over previous
"""Trainium2 Bass kernel for the variants-attention module.

Model (reference):
    q = (x @ Wq)                          [B,N,H,D]
    kv = variants @ Wkv -> k,v            [V,B,N,H,D] each
    attn = softmax(q.k / sqrt(D)) over V  (per-token attention over variants)
    out = (attn.v) @ Wp + bp              [B,N,C]

Strategy: data-parallel over the B*N = 16384 tokens across 8 NeuronCores
(2048 tokens/core), weights replicated.  Host pre-casts inputs to bf16 and
pre-transposes activations to feature-major so the kernel streams them into
the PE array without on-chip transposes.  Projections run on the tensor
engine in bf16 (fp32 PSUM accumulate); the tiny per-token attention over
V=4 variants runs on the vector engine in bf16; softmax exp on the scalar
engine; the attended output is transposed back with PE-transpose and
projected through Wp (+bp folded in as a K=1 matmul row).
"""

import numpy as np
import ml_dtypes

import concourse.bass as bass
import concourse.bacc as bacc
import concourse.tile as tile
from concourse import mybir
from concourse.bass_utils import run_bass_kernel_spmd

# ---------------------------------------------------------------------------

V, B, N, C, H = 4, 4, 4096, 768, 12
D = C // H
SCALE = D**-0.5
TOK = B * N
N_CORES = 8
TPC = TOK // N_CORES  # tokens per core

BF16 = mybir.dt.bfloat16
F32 = mybir.dt.float32
CK = C // 128  # 6 feature chunks

nbf16 = ml_dtypes.bfloat16


def build_nc(tpc=TPC, tile_tok=512, repeat=1, loop=1):
    """Build the per-core Bass program for `tpc` tokens.

    repeat>1 re-runs the whole computation that many times unrolled;
    loop>1 wraps the body in a hardware For_i loop.  Both are idempotent
    (same outputs) and exist only for timing: with loop~1000 the NEFF's
    execution time dominates the axon dispatch jitter, so wall/loop ~= exec.
    """
    assert tpc % tile_tok == 0 and tile_tok % 128 == 0
    n_tiles = tpc // tile_tok
    n_ch = tile_tok // 128  # 128-token chunks per tile

    nc = bacc.Bacc("TRN2", target_bir_lowering=False, debug=False, num_devices=N_CORES)

    xT = nc.dram_tensor("xT", [C, tpc], BF16, kind="ExternalInput").ap()
    pT = nc.dram_tensor("pT", [V, C, tpc], BF16, kind="ExternalInput").ap()
    wq = nc.dram_tensor("wq", [C, C], BF16, kind="ExternalInput").ap()
    wkv = nc.dram_tensor("wkv", [C, 2 * C], BF16, kind="ExternalInput").ap()
    wp = nc.dram_tensor("wp", [C, C], BF16, kind="ExternalInput").ap()
    bp = nc.dram_tensor("bp", [1, C], BF16, kind="ExternalInput").ap()
    ident = nc.dram_tensor("ident", [128, 128], BF16, kind="ExternalInput").ap()
    out = nc.dram_tensor("out", [tpc, C], F32, kind="ExternalOutput").ap()

    xT_v = xT.rearrange("(ck p) t -> p ck t", p=128)
    pT_v = pT.rearrange("v (ck p) t -> p v ck t", p=128)

    with tile.TileContext(nc) as tc:
        with (
            tc.tile_pool(name="const", bufs=1) as constp,
            tc.tile_pool(name="xin", bufs=2) as xin,
            tc.tile_pool(name="pin", bufs=2) as pin,
            tc.tile_pool(name="qkv", bufs=2) as qkvp,
            tc.tile_pool(name="attn", bufs=2) as attp,
            tc.tile_pool(name="outs", bufs=2) as outp,
            tc.tile_pool(name="proj", bufs=2, space="PSUM") as projp,
            tc.tile_pool(name="tps", bufs=2, space="PSUM") as tpsp,
        ):
            # --- persistent constants ---
            # first tile's activations load before the big weight tensors so
            # the PE can start as soon as wq + tile0 land.
            xt0 = xin.tile([128, CK, tile_tok], BF16, tag="xt")
            nc.sync.dma_start(xt0[:], xT_v[:, :, 0:tile_tok])
            pt0 = pin.tile([128, V, CK, tile_tok], BF16, tag="pt")
            nc.sync.dma_start(pt0[:], pT_v[:, :, :, 0:tile_tok])

            wq_sb = constp.tile([128, CK, C], BF16, tag="wq")
            nc.sync.dma_start(wq_sb[:], wq.rearrange("(ck p) o -> p ck o", p=128))
            wkv_sb = constp.tile([128, CK, 2 * C], BF16, tag="wkv")
            nc.sync.dma_start(wkv_sb[:], wkv.rearrange("(ck p) o -> p ck o", p=128))
            wp_sb = constp.tile([128, CK, C], BF16, tag="wp")
            nc.sync.dma_start(wp_sb[:], wp.rearrange("(ck p) o -> p ck o", p=128))
            bp_sb = constp.tile([1, C], BF16, tag="bp")
            nc.sync.dma_start(bp_sb[:], bp[:])
            id_sb = constp.tile([128, 128], BF16, tag="ident")
            nc.sync.dma_start(id_sb[:], ident[:])
            ones_sb = constp.tile([1, 128], BF16, tag="ones")
            nc.gpsimd.memset(ones_sb[:], 1.0)

            n_chunks = n_tiles * n_ch

            def emit_proj(xt, pt, tc_i):
                """q + kv projections for one 128-token chunk -> SBUF bf16."""
                ts = slice(tc_i * 128, (tc_i + 1) * 128)
                q_ps = projp.tile([128, C], F32, tag="proj")
                for ck in range(CK):
                    lhsT = xt[:, ck, ts]
                    nc.tensor.matmul(
                        q_ps[:, 0:512], lhsT, wq_sb[:, ck, 0:512],
                        start=(ck == 0), stop=(ck == CK - 1),
                    )
                    nc.tensor.matmul(
                        q_ps[:, 512:768], lhsT, wq_sb[:, ck, 512:768],
                        start=(ck == 0), stop=(ck == CK - 1),
                    )
                q_sb = qkvp.tile([128, C], BF16, tag="q")
                nc.scalar.copy(q_sb[:], q_ps[:])

                k_sbs, v_sbs = [], []
                for v in range(V):
                    kv_ps = projp.tile([128, 2 * C], F32, tag="proj")
                    for ck in range(CK):
                        lhsT = pt[:, v, ck, ts]
                        for co in range(3):
                            nc.tensor.matmul(
                                kv_ps[:, co * 512 : (co + 1) * 512],
                                lhsT,
                                wkv_sb[:, ck, co * 512 : (co + 1) * 512],
                                start=(ck == 0), stop=(ck == CK - 1),
                            )
                    k_sb = qkvp.tile([128, C], BF16, tag=f"k{v}")
                    v_sb = qkvp.tile([128, C], BF16, tag=f"v{v}")
                    nc.scalar.copy(k_sb[:], kv_ps[:, 0:C])
                    nc.vector.tensor_copy(v_sb[:], kv_ps[:, C : 2 * C])
                    k_sbs.append(k_sb)
                    v_sbs.append(v_sb)
                return q_sb, k_sbs, v_sbs

            def emit_attention(q_sb, k_sbs, v_sbs):
                """per-token attention over V on DVE/ACT -> att [t, C] bf16."""
                L = attp.tile([128, V, H], F32, tag="logits")
                for v in range(V):
                    prod = attp.tile([128, C], BF16, tag="prod")
                    nc.vector.scalar_tensor_tensor(
                        prod[:], q_sb[:], SCALE, k_sbs[v][:],
                        op0=mybir.AluOpType.mult, op1=mybir.AluOpType.mult,
                    )
                    nc.vector.tensor_reduce(
                        L[:, v, :],
                        prod[:].rearrange("p (h d) -> p h d", d=D),
                        axis=mybir.AxisListType.X,
                        op=mybir.AluOpType.add,
                    )

                E = attp.tile([128, V, H], F32, tag="exps")
                nc.scalar.activation(E[:], L[:], mybir.ActivationFunctionType.Exp)
                s01 = attp.tile([128, H], F32, tag="s01")
                s23 = attp.tile([128, H], F32, tag="s23")
                ssum = attp.tile([128, H], F32, tag="ssum")
                nc.vector.tensor_add(s01[:], E[:, 0, :], E[:, 1, :])
                nc.vector.tensor_add(s23[:], E[:, 2, :], E[:, 3, :])
                nc.vector.tensor_add(ssum[:], s01[:], s23[:])
                rcp = attp.tile([128, H], F32, tag="rcp")
                nc.vector.reciprocal(rcp[:], ssum[:])
                W = attp.tile([128, V, H], BF16, tag="wgt")
                for v in range(V):
                    nc.vector.tensor_mul(W[:, v, :], E[:, v, :], rcp[:])

                tmp = []
                for v in range(V):
                    tv = attp.tile([128, C], BF16, tag=f"tv{v}")
                    wb = W[:, v, :].unsqueeze(-1).broadcast_to([128, H, D])
                    nc.vector.tensor_mul(
                        tv[:].rearrange("p (h d) -> p h d", d=D),
                        v_sbs[v][:].rearrange("p (h d) -> p h d", d=D),
                        wb,
                    )
                    tmp.append(tv)
                a01 = attp.tile([128, C], BF16, tag="a01")
                a23 = attp.tile([128, C], BF16, tag="a23")
                att = attp.tile([128, C], BF16, tag="att")
                nc.vector.tensor_add(a01[:], tmp[0][:], tmp[1][:])
                nc.vector.tensor_add(a23[:], tmp[2][:], tmp[3][:])
                nc.vector.tensor_add(att[:], a01[:], a23[:])
                return att

            def emit_output(att, row0):
                """PE-transpose att, project through Wp (+bias), DMA out."""
                tp6 = tpsp.tile([128, CK, 128], BF16, tag="tps")
                for ck in range(CK):
                    nc.tensor.transpose(
                        tp6[:, ck, :], att[:, ck * 128 : (ck + 1) * 128], id_sb[:]
                    )
                attT = outp.tile([128, CK, 128], BF16, tag="attT")
                nc.vector.tensor_copy(attT[:], tp6[:])

                o_ps = projp.tile([128, C], F32, tag="proj")
                for ck in range(CK):
                    lhsT = attT[:, ck, :]
                    nc.tensor.matmul(
                        o_ps[:, 0:512], lhsT, wp_sb[:, ck, 0:512],
                        start=(ck == 0), stop=False,
                    )
                    nc.tensor.matmul(
                        o_ps[:, 512:768], lhsT, wp_sb[:, ck, 512:768],
                        start=(ck == 0), stop=False,
                    )
                nc.tensor.matmul(
                    o_ps[:, 0:512], ones_sb[:], bp_sb[:, 0:512],
                    start=False, stop=True,
                )
                nc.tensor.matmul(
                    o_ps[:, 512:768], ones_sb[:], bp_sb[:, 512:768],
                    start=False, stop=True,
                )
                o_sb = outp.tile([128, C], F32, tag="osb")
                nc.scalar.copy(o_sb[:], o_ps[:])
                nc.sync.dma_start(out[row0 : row0 + 128, :], o_sb[:])

            def emit_body(rep):
                # software pipeline: emit the PE-dependent tail (transpose +
                # out-proj) one chunk behind, so PE never waits on the DVE
                # attention chain.
                pending = None  # (att, row0)
                xt = pt = None
                for ci in range(n_chunks + 1):
                    if ci < n_chunks:
                        it, tc_i = divmod(ci, n_ch)
                        if tc_i == 0:
                            t0 = it * tile_tok
                            if rep == 0 and it == 0:
                                xt, pt = xt0, pt0
                            else:
                                xt = xin.tile([128, CK, tile_tok], BF16, tag="xt")
                                nc.sync.dma_start(
                                    xt[:], xT_v[:, :, t0 : t0 + tile_tok]
                                )
                                pt = pin.tile([128, V, CK, tile_tok], BF16, tag="pt")
                                nc.sync.dma_start(
                                    pt[:], pT_v[:, :, :, t0 : t0 + tile_tok]
                                )
                        q_sb, k_sbs, v_sbs = emit_proj(xt, pt, tc_i)
                        att = emit_attention(q_sb, k_sbs, v_sbs)
                        next_pending = (att, it * tile_tok + tc_i * 128)
                    else:
                        next_pending = None
                    if pending is not None:
                        emit_output(*pending)
                    pending = next_pending

            if loop > 1:
                with tc.For_i(0, loop, 1):
                    emit_body(1)
            else:
                for rep in range(repeat):
                    emit_body(rep)

    nc.compile()
    return nc


def _prep_inputs(x, variants_patches, Wq, Wkv, Wp, bp):
    """Host-side: cast to bf16, transpose activations feature-major, shard."""
    xs = np.ascontiguousarray(x.reshape(TOK, C).T.astype(nbf16))  # [C, TOK]
    ps = np.ascontiguousarray(
        variants_patches.reshape(V, TOK, C).transpose(0, 2, 1).astype(nbf16)
    )  # [V, C, TOK]
    wq = np.ascontiguousarray(Wq.astype(nbf16))
    wkv = np.ascontiguousarray(Wkv.astype(nbf16))
    wp = np.ascontiguousarray(Wp.astype(nbf16))
    bpb = np.ascontiguousarray(bp.reshape(1, C).astype(nbf16))
    ident = np.eye(128, dtype=nbf16)

    in_maps = []
    for c in range(N_CORES):
        sl = slice(c * TPC, (c + 1) * TPC)
        in_maps.append(
            {
                "xT": np.ascontiguousarray(xs[:, sl]),
                "pT": np.ascontiguousarray(ps[:, :, sl]),
                "wq": wq,
                "wkv": wkv,
                "wp": wp,
                "bp": bpb,
                "ident": ident,
            }
        )
    return in_maps


_NC_CACHE = {}


def run(x, variants_patches, Wq, Wkv, Wp, bp, **spmd_kwargs):
    if "nc" not in _NC_CACHE:
        _NC_CACHE["nc"] = build_nc()
    nc = _NC_CACHE["nc"]
    in_maps = _prep_inputs(x, variants_patches, Wq, Wkv, Wp, bp)
    res = run_bass_kernel_spmd(nc, in_maps, core_ids=list(range(N_CORES)), **spmd_kwargs)
    full = np.concatenate([res.results[c]["out"] for c in range(N_CORES)], axis=0)
    return full.reshape(B, N, C), res


def make_runner(nc, in_maps):
    """Compile the SPMD NEFF via the PJRT path; return (run_fn, collect_fn).

    run_fn() executes once (blocking) and returns the raw jax outputs;
    collect_fn(out) converts to per-core result dicts.  Inputs live on
    device; each call re-donates freshly-uploaded zero output buffers.
    """
    import jax
    import time
    from jax.sharding import Mesh, PartitionSpec
    from jax.experimental.shard_map import shard_map
    from concourse import bass2jax, mybir as _mybir
    from concourse.bass2jax import _bass_exec_p, install_neuronx_cc_hook

    install_neuronx_cc_hook()
    n_cores = len(in_maps)
    partition_name = nc.partition_id_tensor.name if nc.partition_id_tensor else None

    in_names, out_names, out_avals, zero_outs = [], [], [], []
    for alloc in nc.m.functions[0].allocations:
        if not isinstance(alloc, _mybir.MemoryLocationSet):
            continue
        name = alloc.memorylocations[0].name
        if alloc.kind == "ExternalInput":
            if name != partition_name:
                in_names.append(name)
        elif alloc.kind == "ExternalOutput":
            shape = tuple(alloc.tensor_shape)
            dtype = _mybir.dt.np(alloc.dtype)
            out_names.append(name)
            out_avals.append(jax.core.ShapedArray(shape, dtype))
            zero_outs.append(np.zeros(shape, dtype))
    n_params = len(in_names)
    n_outs = len(out_avals)
    in_names_all = in_names + out_names
    if partition_name is not None:
        in_names_all.append(partition_name)

    def _body(*args):
        operands = list(args)
        if partition_name is not None:
            operands.append(bass2jax.partition_id_tensor())
        outs = _bass_exec_p.bind(
            *operands,
            out_avals=tuple(out_avals),
            in_names=tuple(in_names_all),
            out_names=tuple(out_names),
            lowering_input_output_aliases=(),
            sim_require_finite=True,
            sim_require_nnan=True,
            nc=nc,
        )
        return tuple(outs)

    devices = jax.devices()[:n_cores]
    mesh = Mesh(np.asarray(devices), ("core",))
    donate = tuple(range(n_params, n_params + n_outs))
    sharded = jax.jit(
        shard_map(
            _body, mesh=mesh,
            in_specs=(PartitionSpec("core"),) * (n_params + n_outs),
            out_specs=(PartitionSpec("core"),) * n_outs,
            check_rep=False,
        ),
        donate_argnums=donate, keep_unused=True,
    )
    sh = jax.sharding.NamedSharding(mesh, PartitionSpec("core"))
    concat_in = [
        jax.device_put(
            np.concatenate([np.asarray(in_maps[c][nm]) for c in range(n_cores)], axis=0),
            sh,
        )
        for nm in in_names
    ]
    def fresh_zeros():
        return [
            jax.device_put(np.zeros((n_cores * z.shape[0], *z.shape[1:]), z.dtype), sh)
            for z in zero_outs
        ]

    def run_fn():
        zs = fresh_zeros()
        jax.block_until_ready(zs)
        t0 = time.perf_counter()
        out = sharded(*concat_in, *zs)
        jax.block_until_ready(out)
        return time.perf_counter() - t0, out

    def collect_fn(out):
        return [
            {nm: np.asarray(out[i]).reshape(n_cores, *out_avals[i].shape)[c]
             for i, nm in enumerate(out_names)}
            for c in range(n_cores)
        ]

    return run_fn, collect_fn


def bench(nc, in_maps, iters=20):
    run_fn, collect_fn = make_runner(nc, in_maps)
    run_fn()  # warmup/compile
    times = []
    out = None
    for _ in range(iters):
        dt, out = run_fn()
        times.append(dt)
    return times, collect_fn(out)


def kernel(x, variants_patches, num_layer=None, Wq=None, Wkv=None, Wp=None, bp=None):
    x = np.asarray(x, dtype=np.float32)
    variants_patches = np.asarray(variants_patches, dtype=np.float32)
    Wq = np.asarray(Wq, dtype=np.float32)
    Wkv = np.asarray(Wkv, dtype=np.float32)
    Wp = np.asarray(Wp, dtype=np.float32)
    bp = np.asarray(bp, dtype=np.float32)
    out, _ = run(x, variants_patches, Wq, Wkv, Wp, bp)
    return out


# revision 25
# speedup vs baseline: 1.7841x; 1.1535x over previous
"""Trainium2 Bass kernel for the variants-attention module.

Model (reference):
    q = (x @ Wq)                          [B,N,H,D]
    kv = variants @ Wkv -> k,v            [V,B,N,H,D] each
    attn = softmax(q.k / sqrt(D)) over V  (per-token attention over variants)
    out = (attn.v) @ Wp + bp              [B,N,C]

Strategy: data-parallel over the B*N = 16384 tokens across 8 NeuronCores
(2048 tokens/core), weights replicated.  Host pre-casts inputs to bf16 and
pre-transposes activations to feature-major so the kernel streams them into
the PE array without on-chip transposes.  Projections run on the tensor
engine in bf16 (fp32 PSUM accumulate); the tiny per-token attention over
V=4 variants runs on the vector engine in bf16; softmax exp on the scalar
engine; the attended output is transposed back with PE-transpose and
projected through Wp (+bp folded in as a K=1 matmul row).
"""

import numpy as np
import ml_dtypes

import concourse.bass as bass
import concourse.bacc as bacc
import concourse.tile as tile
from concourse import mybir
from concourse.bass_utils import run_bass_kernel_spmd

# ---------------------------------------------------------------------------

V, B, N, C, H = 4, 4, 4096, 768, 12
D = C // H
SCALE = D**-0.5
TOK = B * N
N_CORES = 8
TPC = TOK // N_CORES  # tokens per core

BF16 = mybir.dt.bfloat16
F32 = mybir.dt.float32
CK = C // 128  # 6 feature chunks

nbf16 = ml_dtypes.bfloat16


def build_nc(tpc=TPC, tile_tok=512, repeat=1, loop=1, ablate=None):
    """Build the per-core Bass program for `tpc` tokens.

    repeat>1 re-runs the whole computation that many times unrolled;
    loop>1 wraps the body in a hardware For_i loop.  Both are idempotent
    (same outputs) and exist only for timing: with loop~1000 the NEFF's
    execution time dominates the axon dispatch jitter, so wall/loop ~= exec.
    """
    assert tpc % tile_tok == 0 and tile_tok % 128 == 0
    n_tiles = tpc // tile_tok
    n_ch = tile_tok // 128  # 128-token chunks per tile

    nc = bacc.Bacc("TRN2", target_bir_lowering=False, debug=False, num_devices=N_CORES)

    xT = nc.dram_tensor("xT", [C, tpc], BF16, kind="ExternalInput").ap()
    pT = nc.dram_tensor("pT", [V, C, tpc], BF16, kind="ExternalInput").ap()
    wq = nc.dram_tensor("wq", [C, C], BF16, kind="ExternalInput").ap()
    wkv = nc.dram_tensor("wkv", [C, 2 * C], BF16, kind="ExternalInput").ap()
    wp = nc.dram_tensor("wp", [C, C], BF16, kind="ExternalInput").ap()
    bp = nc.dram_tensor("bp", [1, C], BF16, kind="ExternalInput").ap()
    ident = nc.dram_tensor("ident", [128, 128], BF16, kind="ExternalInput").ap()
    out = nc.dram_tensor("out", [tpc, C], F32, kind="ExternalOutput").ap()

    xT_v = xT.rearrange("(ck p) t -> p ck t", p=128)
    pT_v = pT.rearrange("v (ck p) t -> p v ck t", p=128)

    with tile.TileContext(nc) as tc:
        with (
            tc.tile_pool(name="const", bufs=1) as constp,
            tc.tile_pool(name="xin", bufs=2) as xin,
            tc.tile_pool(name="pin", bufs=2) as pin,
            tc.tile_pool(name="qkv", bufs=2) as qkvp,
            tc.tile_pool(name="attn", bufs=2) as attp,
            tc.tile_pool(name="outs", bufs=2) as outp,
            tc.tile_pool(name="proj", bufs=2, space="PSUM") as projp,
            tc.tile_pool(name="tps", bufs=2, space="PSUM") as tpsp,
        ):
            # --- persistent constants ---
            # first tile's activations load before the big weight tensors so
            # the PE can start as soon as wq + tile0 land.
            xt0 = xin.tile([128, CK, tile_tok], BF16, tag="xt")
            nc.sync.dma_start(xt0[:], xT_v[:, :, 0:tile_tok])
            pt0 = pin.tile([128, V, CK, tile_tok], BF16, tag="pt")
            nc.sync.dma_start(pt0[:], pT_v[:, :, :, 0:tile_tok])

            wq_sb = constp.tile([128, CK, C], BF16, tag="wq")
            nc.sync.dma_start(wq_sb[:], wq.rearrange("(ck p) o -> p ck o", p=128))
            wkv_sb = constp.tile([128, CK, 2 * C], BF16, tag="wkv")
            nc.sync.dma_start(wkv_sb[:], wkv.rearrange("(ck p) o -> p ck o", p=128))
            wp_sb = constp.tile([128, CK, C], BF16, tag="wp")
            nc.sync.dma_start(wp_sb[:], wp.rearrange("(ck p) o -> p ck o", p=128))
            bp_sb = constp.tile([1, C], BF16, tag="bp")
            nc.sync.dma_start(bp_sb[:], bp[:])
            id_sb = constp.tile([128, 128], BF16, tag="ident")
            nc.sync.dma_start(id_sb[:], ident[:])
            ones_sb = constp.tile([1, 128], BF16, tag="ones")
            nc.gpsimd.memset(ones_sb[:], 1.0)

            n_chunks = n_tiles * n_ch

            def emit_proj(xt, pt, tc_i):
                """q + kv projections for one 128-token chunk -> SBUF bf16."""
                ts = slice(tc_i * 128, (tc_i + 1) * 128)
                q_ps = projp.tile([128, C], F32, tag="proj")
                for ck in range(CK):
                    lhsT = xt[:, ck, ts]
                    nc.tensor.matmul(
                        q_ps[:, 0:512], lhsT, wq_sb[:, ck, 0:512],
                        start=(ck == 0), stop=(ck == CK - 1),
                    )
                    nc.tensor.matmul(
                        q_ps[:, 512:768], lhsT, wq_sb[:, ck, 512:768],
                        start=(ck == 0), stop=(ck == CK - 1),
                    )
                q_sb = qkvp.tile([128, C], BF16, tag="q")
                nc.scalar.copy(q_sb[:], q_ps[:])

                k_sbs, v_sbs = [], []
                for v in range(V):
                    kv_ps = projp.tile([128, 2 * C], F32, tag="proj")
                    for ck in range(CK):
                        lhsT = pt[:, v, ck, ts]
                        for co in range(3):
                            nc.tensor.matmul(
                                kv_ps[:, co * 512 : (co + 1) * 512],
                                lhsT,
                                wkv_sb[:, ck, co * 512 : (co + 1) * 512],
                                start=(ck == 0), stop=(ck == CK - 1),
                            )
                    k_sb = qkvp.tile([128, C], BF16, tag=f"k{v}")
                    v_sb = qkvp.tile([128, C], BF16, tag=f"v{v}")
                    nc.scalar.copy(k_sb[:], kv_ps[:, 0:C])
                    nc.vector.tensor_copy(v_sb[:], kv_ps[:, C : 2 * C])
                    k_sbs.append(k_sb)
                    v_sbs.append(v_sb)
                return q_sb, k_sbs, v_sbs

            def emit_attention(q_sb, k_sbs, v_sbs):
                """per-token attention over V on DVE/ACT -> att [t, C] bf16."""
                L = attp.tile([128, V, H], F32, tag="logits")
                for v in range(V):
                    prod = attp.tile([128, C], BF16, tag="prod")
                    nc.vector.scalar_tensor_tensor(
                        prod[:], q_sb[:], SCALE, k_sbs[v][:],
                        op0=mybir.AluOpType.mult, op1=mybir.AluOpType.mult,
                    )
                    nc.vector.tensor_reduce(
                        L[:, v, :],
                        prod[:].rearrange("p (h d) -> p h d", d=D),
                        axis=mybir.AxisListType.X,
                        op=mybir.AluOpType.add,
                    )

                E = attp.tile([128, V, H], F32, tag="exps")
                nc.scalar.activation(E[:], L[:], mybir.ActivationFunctionType.Exp)
                s01 = attp.tile([128, H], F32, tag="s01")
                s23 = attp.tile([128, H], F32, tag="s23")
                ssum = attp.tile([128, H], F32, tag="ssum")
                nc.vector.tensor_add(s01[:], E[:, 0, :], E[:, 1, :])
                nc.vector.tensor_add(s23[:], E[:, 2, :], E[:, 3, :])
                nc.vector.tensor_add(ssum[:], s01[:], s23[:])
                rcp = attp.tile([128, H], F32, tag="rcp")
                nc.vector.reciprocal(rcp[:], ssum[:])
                W = attp.tile([128, V, H], BF16, tag="wgt")
                for v in range(V):
                    nc.vector.tensor_mul(W[:, v, :], E[:, v, :], rcp[:])

                tmp = []
                for v in range(V):
                    tv = attp.tile([128, C], BF16, tag=f"tv{v}")
                    wb = W[:, v, :].unsqueeze(-1).broadcast_to([128, H, D])
                    nc.vector.tensor_mul(
                        tv[:].rearrange("p (h d) -> p h d", d=D),
                        v_sbs[v][:].rearrange("p (h d) -> p h d", d=D),
                        wb,
                    )
                    tmp.append(tv)
                a01 = attp.tile([128, C], BF16, tag="a01")
                a23 = attp.tile([128, C], BF16, tag="a23")
                att = attp.tile([128, C], BF16, tag="att")
                nc.vector.tensor_add(a01[:], tmp[0][:], tmp[1][:])
                nc.vector.tensor_add(a23[:], tmp[2][:], tmp[3][:])
                nc.vector.tensor_add(att[:], a01[:], a23[:])
                return att

            def emit_output(att, row0):
                """DMA-transpose att, project through Wp (+bias), DMA out."""
                attT = outp.tile([128, CK, 128], BF16, tag="attT")
                for ck in range(CK):
                    nc.sync.dma_start_transpose(
                        attT[:, ck, :], att[:, ck * 128 : (ck + 1) * 128]
                    )

                o_ps = projp.tile([128, C], F32, tag="proj")
                for ck in range(CK):
                    lhsT = attT[:, ck, :]
                    nc.tensor.matmul(
                        o_ps[:, 0:512], lhsT, wp_sb[:, ck, 0:512],
                        start=(ck == 0), stop=False,
                    )
                    nc.tensor.matmul(
                        o_ps[:, 512:768], lhsT, wp_sb[:, ck, 512:768],
                        start=(ck == 0), stop=False,
                    )
                nc.tensor.matmul(
                    o_ps[:, 0:512], ones_sb[:], bp_sb[:, 0:512],
                    start=False, stop=True,
                )
                nc.tensor.matmul(
                    o_ps[:, 512:768], ones_sb[:], bp_sb[:, 512:768],
                    start=False, stop=True,
                )
                o_sb = outp.tile([128, C], F32, tag="osb")
                nc.scalar.copy(o_sb[:], o_ps[:])
                nc.sync.dma_start(out[row0 : row0 + 128, :], o_sb[:])

            def emit_body(rep):
                # software pipeline: emit the PE-dependent tail (transpose +
                # out-proj) one chunk behind, so PE never waits on the DVE
                # attention chain.
                pending = None  # (att, row0)
                xt = pt = None
                for ci in range(n_chunks + 1):
                    if ci < n_chunks:
                        it, tc_i = divmod(ci, n_ch)
                        if tc_i == 0:
                            t0 = it * tile_tok
                            if rep == 0 and it == 0:
                                xt, pt = xt0, pt0
                            else:
                                xt = xin.tile([128, CK, tile_tok], BF16, tag="xt")
                                nc.sync.dma_start(
                                    xt[:], xT_v[:, :, t0 : t0 + tile_tok]
                                )
                                pt = pin.tile([128, V, CK, tile_tok], BF16, tag="pt")
                                nc.sync.dma_start(
                                    pt[:], pT_v[:, :, :, t0 : t0 + tile_tok]
                                )
                        q_sb, k_sbs, v_sbs = emit_proj(xt, pt, tc_i)
                        if ablate == "noattn":
                            att = q_sb
                        else:
                            att = emit_attention(q_sb, k_sbs, v_sbs)
                        next_pending = (att, it * tile_tok + tc_i * 128)
                    else:
                        next_pending = None
                    if pending is not None:
                        emit_output(*pending)
                    pending = next_pending

            if loop > 1:
                with tc.For_i(0, loop, 1):
                    emit_body(1)
            else:
                for rep in range(repeat):
                    emit_body(rep)

    nc.compile()
    return nc


def _prep_inputs(x, variants_patches, Wq, Wkv, Wp, bp):
    """Host-side: cast to bf16, transpose activations feature-major, shard."""
    xs = np.ascontiguousarray(x.reshape(TOK, C).T.astype(nbf16))  # [C, TOK]
    ps = np.ascontiguousarray(
        variants_patches.reshape(V, TOK, C).transpose(0, 2, 1).astype(nbf16)
    )  # [V, C, TOK]
    wq = np.ascontiguousarray(Wq.astype(nbf16))
    wkv = np.ascontiguousarray(Wkv.astype(nbf16))
    wp = np.ascontiguousarray(Wp.astype(nbf16))
    bpb = np.ascontiguousarray(bp.reshape(1, C).astype(nbf16))
    ident = np.eye(128, dtype=nbf16)

    in_maps = []
    for c in range(N_CORES):
        sl = slice(c * TPC, (c + 1) * TPC)
        in_maps.append(
            {
                "xT": np.ascontiguousarray(xs[:, sl]),
                "pT": np.ascontiguousarray(ps[:, :, sl]),
                "wq": wq,
                "wkv": wkv,
                "wp": wp,
                "bp": bpb,
                "ident": ident,
            }
        )
    return in_maps


_NC_CACHE = {}


def run(x, variants_patches, Wq, Wkv, Wp, bp, **spmd_kwargs):
    if "nc" not in _NC_CACHE:
        _NC_CACHE["nc"] = build_nc()
    nc = _NC_CACHE["nc"]
    in_maps = _prep_inputs(x, variants_patches, Wq, Wkv, Wp, bp)
    res = run_bass_kernel_spmd(nc, in_maps, core_ids=list(range(N_CORES)), **spmd_kwargs)
    full = np.concatenate([res.results[c]["out"] for c in range(N_CORES)], axis=0)
    return full.reshape(B, N, C), res


def make_runner(nc, in_maps):
    """Compile the SPMD NEFF via the PJRT path; return (run_fn, collect_fn).

    run_fn() executes once (blocking) and returns the raw jax outputs;
    collect_fn(out) converts to per-core result dicts.  Inputs live on
    device; each call re-donates freshly-uploaded zero output buffers.
    """
    import jax
    import time
    from jax.sharding import Mesh, PartitionSpec
    from jax.experimental.shard_map import shard_map
    from concourse import bass2jax, mybir as _mybir
    from concourse.bass2jax import _bass_exec_p, install_neuronx_cc_hook

    install_neuronx_cc_hook()
    n_cores = len(in_maps)
    partition_name = nc.partition_id_tensor.name if nc.partition_id_tensor else None

    in_names, out_names, out_avals, zero_outs = [], [], [], []
    for alloc in nc.m.functions[0].allocations:
        if not isinstance(alloc, _mybir.MemoryLocationSet):
            continue
        name = alloc.memorylocations[0].name
        if alloc.kind == "ExternalInput":
            if name != partition_name:
                in_names.append(name)
        elif alloc.kind == "ExternalOutput":
            shape = tuple(alloc.tensor_shape)
            dtype = _mybir.dt.np(alloc.dtype)
            out_names.append(name)
            out_avals.append(jax.core.ShapedArray(shape, dtype))
            zero_outs.append(np.zeros(shape, dtype))
    n_params = len(in_names)
    n_outs = len(out_avals)
    in_names_all = in_names + out_names
    if partition_name is not None:
        in_names_all.append(partition_name)

    def _body(*args):
        operands = list(args)
        if partition_name is not None:
            operands.append(bass2jax.partition_id_tensor())
        outs = _bass_exec_p.bind(
            *operands,
            out_avals=tuple(out_avals),
            in_names=tuple(in_names_all),
            out_names=tuple(out_names),
            lowering_input_output_aliases=(),
            sim_require_finite=True,
            sim_require_nnan=True,
            nc=nc,
        )
        return tuple(outs)

    devices = jax.devices()[:n_cores]
    mesh = Mesh(np.asarray(devices), ("core",))
    donate = tuple(range(n_params, n_params + n_outs))
    sharded = jax.jit(
        shard_map(
            _body, mesh=mesh,
            in_specs=(PartitionSpec("core"),) * (n_params + n_outs),
            out_specs=(PartitionSpec("core"),) * n_outs,
            check_rep=False,
        ),
        donate_argnums=donate, keep_unused=True,
    )
    sh = jax.sharding.NamedSharding(mesh, PartitionSpec("core"))
    concat_in = [
        jax.device_put(
            np.concatenate([np.asarray(in_maps[c][nm]) for c in range(n_cores)], axis=0),
            sh,
        )
        for nm in in_names
    ]
    def fresh_zeros():
        return [
            jax.device_put(np.zeros((n_cores * z.shape[0], *z.shape[1:]), z.dtype), sh)
            for z in zero_outs
        ]

    def run_fn():
        zs = fresh_zeros()
        jax.block_until_ready(zs)
        t0 = time.perf_counter()
        out = sharded(*concat_in, *zs)
        jax.block_until_ready(out)
        return time.perf_counter() - t0, out

    def collect_fn(out):
        return [
            {nm: np.asarray(out[i]).reshape(n_cores, *out_avals[i].shape)[c]
             for i, nm in enumerate(out_names)}
            for c in range(n_cores)
        ]

    return run_fn, collect_fn


def bench(nc, in_maps, iters=20):
    run_fn, collect_fn = make_runner(nc, in_maps)
    run_fn()  # warmup/compile
    times = []
    out = None
    for _ in range(iters):
        dt, out = run_fn()
        times.append(dt)
    return times, collect_fn(out)


def kernel(x, variants_patches, num_layer=None, Wq=None, Wkv=None, Wp=None, bp=None):
    x = np.asarray(x, dtype=np.float32)
    variants_patches = np.asarray(variants_patches, dtype=np.float32)
    Wq = np.asarray(Wq, dtype=np.float32)
    Wkv = np.asarray(Wkv, dtype=np.float32)
    Wp = np.asarray(Wp, dtype=np.float32)
    bp = np.asarray(bp, dtype=np.float32)
    out, _ = run(x, variants_patches, Wq, Wkv, Wp, bp)
    return out
